# revision 1
# baseline (speedup 1.0000x reference)
"""Multi-head attention forward on 8 TRN2 NeuronCores.

Sharding: tensor-parallel over heads (4 groups of 4 heads) x data-parallel
over batch (2). Core c: batch c//4, heads [4*(c%4), 4*(c%4)+4).
Each 4-core batch group ReduceScatters the projection partials (bf16, 8
chunks, overlapped with compute) so every core ends with disjoint
[8, 64, 1024] slices of the final output; the host reassembles.

Compute layout is feature-major (transposed) throughout:
  qkvT = W_shard^T @ x^T          [768, T]   (PE, bf16 in / f32 psum)
  S^T  = kT^T qT per k-tile pair  [128, 1024] psum (two 512-col halves)
  P^T  = exp(S^T / 64)            (ScalarE, 1024 wide; no max-subtraction
                                   needed: scores have sigma ~0.125)
  O_aug^T = V_aug^T @ P^T accum   [65, 512]  (V_aug has a ones column so
                                   row 64 accumulates the softmax denom)
  transpose 128-col blocks of O_aug -> q on partitions -> reciprocal *
  scale on VectorE -> transpose back -> O_all^T
  y = O_all^T^T @ W_proj          [128, 512] psum tiles

The S->exp->O chain is software-pipelined: the next pair's S matmuls are
emitted before the previous pair's O matmuls so the in-order PE queue
never waits head-of-line on ScalarE's exp.
"""
import os
import sys
import types

import numpy as np

if "/opt/trn_rl_repo" not in sys.path:
    sys.path.insert(0, "/opt/trn_rl_repo")

import concourse.bass as bass
import concourse.bacc as bacc
import concourse.tile as tile
import concourse.mybir as mybir
from concourse import masks
from concourse.bass_utils import run_bass_kernel_spmd

B, T, D = 2, 2048, 1024
H, HD = 16, 64
N_CORES = 8
GROUPS = [[0, 1, 2, 3], [4, 5, 6, 7]]
HPC = 4                 # heads per core
DSH = HPC * HD          # 256 per-core head features
QKV_COLS = 3 * DSH      # 768
TQC = 512               # q-chunk
N_TQ = T // TQC         # 4
N_KT = T // 128         # 16 k-tiles
# reduce-scatter chunks (row_base, rows): coarse early, fine at the tail so
# the last exposed collective is small
RS_CHUNKS = [(256 * i, 256) for i in range(8)]

f32 = mybir.dt.float32
bf16 = mybir.dt.bfloat16

LAST_EXEC_NS = None
_CACHE = {}


def _build():
    nc = bacc.Bacc("TRN2", target_bir_lowering=False, debug=False,
                   num_devices=N_CORES)
    xT_ext = nc.dram_tensor("xT", [D, T], bf16, kind="ExternalInput")
    wqkv_ext = nc.dram_tensor("w_qkv", [D, QKV_COLS], bf16,
                              kind="ExternalInput")
    wproj_ext = nc.dram_tensor("w_proj", [DSH, D], bf16, kind="ExternalInput")
    out_ext = nc.dram_tensor("out", [T // 4, D], bf16, kind="ExternalOutput")
    Exp = mybir.ActivationFunctionType.Exp

    with tile.TileContext(nc) as tc:
        with (
            tc.tile_pool(name="persist", bufs=1) as persist,
            tc.tile_pool(name="dram", bufs=1, space="DRAM") as drampool,
        ):
            qkvT = persist.tile([128, 4, T], bf16)   # q,k rows m*128+p of [512,T]
            wproj = persist.tile([128, 2, D], bf16)  # rows of [256, 1024]
            oallT = persist.tile([128, 2, T], bf16)  # rows of O_all^T [256, T]
            vaug = persist.tile([128, HPC, N_KT, 80], bf16)

            nc.sync.dma_start(
                wproj[:], wproj_ext.ap().rearrange("(c p) d -> p c d", p=128))

            y_bounce = drampool.tile([T, D], bf16, tag="ybounce")

            # ---- inputs (SBUF-resident through the whole kernel) ----
            with (
                tc.tile_pool(name="qkv_in", bufs=1) as qin,
            ):
                xT = qin.tile([128, 8, T], bf16)
                wqkv = qin.tile([128, 8, QKV_COLS], bf16)
                nc.sync.dma_start(
                    wqkv[:], wqkv_ext.ap().rearrange("(k p) m -> p k m", p=128))
                xT_src = xT_ext.ap().rearrange("(k p) t -> p k t", p=128)
                for tch in range(N_TQ):
                    t0 = tch * TQC
                    nc.sync.dma_start(xT[:, :, t0:t0 + TQC],
                                      xT_src[:, :, t0:t0 + TQC])
                nc.gpsimd.memset(vaug[:], 1.0)

                def qkv_m(m, pool):
                    """One 128-col block of q/k rows: qkvT[:, m, :]."""
                    for tch in range(N_TQ):
                        t0 = tch * TQC
                        ps = pool.tile([128, TQC], f32, tag="qkv")
                        for k in range(8):
                            nc.tensor.matmul(
                                ps[:],
                                wqkv[:, k, m * 128:(m + 1) * 128],
                                xT[:, k, t0:t0 + TQC],
                                start=(k == 0), stop=(k == 7),
                            )
                        nc.vector.tensor_copy(qkvT[:, m, t0:t0 + TQC], ps[:])

                # ---- phase A: q,k rows for heads 0/1 + V for all heads ----
                with tc.tile_pool(name="ps_a", bufs=2, space="PSUM") as psA:
                    qkv_m(0, psA)
                    qkv_m(2, psA)
                    # V directly in [T, hd] orientation (x tile stationary)
                    for kt in range(N_KT):
                        vps = psA.tile([128, DSH], f32, tag="v")
                        for k in range(8):
                            nc.tensor.matmul(
                                vps[:],
                                xT[:, k, kt * 128:(kt + 1) * 128],
                                wqkv[:, k, 2 * DSH:3 * DSH],
                                start=(k == 0), stop=(k == 7),
                            )
                        nc.vector.tensor_copy(
                            vaug[:, :, kt, 0:HD],
                            vps[:].rearrange("p (h d) -> p h d", d=HD))

                # ---- attention + interleaved QKV(m=1,3) + proj + RS ----
                with (
                    tc.tile_pool(name="ps_s", bufs=2, space="PSUM") as pss,
                    tc.tile_pool(name="ps_o", bufs=1, space="PSUM") as pso,
                    tc.tile_pool(name="ps_y", bufs=2, space="PSUM") as psy,
                    tc.tile_pool(name="ps_qb", bufs=1, space="PSUM") as psqB,
                    tc.tile_pool(name="attn", bufs=3) as apool,
                    tc.tile_pool(name="attn2", bufs=2) as apool2,
                ):
                    def epilogue(tq, h, o_sb):
                        """Normalize head h's O (feature-major) into oallT:
                        broadcast the ones-row across partitions (GpSimd),
                        parallel reciprocal + multiply on DVE. No transposes.
                        """
                        q0 = tq * TQC
                        om, op = h // 2, (h % 2) * 64
                        rrow = apool2.tile([1, TQC], f32, tag="rrow")
                        nc.vector.tensor_copy(rrow[:], o_sb[HD:HD + 1, :])
                        rb = apool2.tile([HD, TQC], f32, tag="rb")
                        nc.gpsimd.partition_broadcast(rb[:], rrow[:])
                        rbinv = apool2.tile([HD, TQC], f32, tag="rbinv")
                        nc.vector.reciprocal(rbinv[:], rb[:])
                        nc.vector.tensor_tensor(
                            out=oallT[op:op + HD, om, q0:q0 + TQC],
                            in0=o_sb[0:HD, :], in1=rbinv[:],
                            op=mybir.AluOpType.mult)

                    def attn_unit(tq, h):
                        q0 = tq * TQC
                        qm, qp = h // 2, (h % 2) * 64
                        km = 2 + h // 2
                        o_ps = pso.tile([HD + 1, TQC], f32, tag="o")
                        prev_p = None
                        for j in range(N_KT // 2):   # k-tile pairs
                            s2 = pss.tile([128, 2 * TQC], f32, tag="s")
                            for half in range(2):
                                kt = 2 * j + half
                                nc.tensor.matmul(
                                    s2[:, half * TQC:(half + 1) * TQC],
                                    qkvT[qp:qp + HD, km,
                                         kt * 128:(kt + 1) * 128],
                                    qkvT[qp:qp + HD, qm, q0:q0 + TQC],
                                    start=True, stop=True,
                                )
                            p2 = apool.tile([128, 2 * TQC], bf16, tag="p")
                            nc.scalar.activation(p2[:], s2[:], Exp,
                                                 scale=1.0 / HD)
                            if prev_p is not None:
                                pj, pp = prev_p
                                for half in range(2):
                                    kt = 2 * pj + half
                                    nc.tensor.matmul(
                                        o_ps[:], vaug[:, h, kt, 0:HD + 1],
                                        pp[:, half * TQC:(half + 1) * TQC],
                                        start=(kt == 0), stop=False,
                                    )
                            prev_p = (j, p2)
                        pj, pp = prev_p
                        for half in range(2):
                            kt = 2 * pj + half
                            nc.tensor.matmul(
                                o_ps[:], vaug[:, h, kt, 0:HD + 1],
                                pp[:, half * TQC:(half + 1) * TQC],
                                start=False, stop=(kt == N_KT - 1),
                            )
                        o_sb = apool.tile([HD + 1, TQC], bf16, tag="osb")
                        nc.vector.tensor_copy(o_sb[:], o_ps[:])
                        return o_sb

                    pending = None
                    rs_next = [0, 0]
                    for tq in range(N_TQ):
                        q0 = tq * TQC
                        for h in range(HPC):
                            o_sb = attn_unit(tq, h)
                            if pending is not None:
                                epilogue(*pending)
                                pending = None
                            if h < HPC - 1:
                                pending = (tq, h, o_sb)
                            else:
                                epilogue(tq, h, o_sb)
                            # interleave the remaining q/k row blocks as PE
                            # filler inside the first chunk's attention
                            if tq == 0 and h == 0:
                                qkv_m(1, psqB)
                            if tq == 0 and h == 1:
                                qkv_m(3, psqB)
                        # proj for this chunk
                        for tt in range(TQC // 128):
                            y_sb = apool.tile([128, D], bf16, tag="ysb")
                            for nn in range(2):
                                y_ps = psy.tile([128, 512], f32, tag="y")
                                for kc in range(2):
                                    nc.tensor.matmul(
                                        y_ps[:],
                                        oallT[:, kc, q0 + tt * 128:
                                              q0 + (tt + 1) * 128],
                                        wproj[:, kc,
                                              nn * 512:(nn + 1) * 512],
                                        start=(kc == 0), stop=(kc == 1),
                                    )
                                nc.vector.tensor_copy(
                                    y_sb[:, nn * 512:(nn + 1) * 512], y_ps[:])
                            nc.sync.dma_start(
                                y_bounce[q0 + tt * 128:q0 + (tt + 1) * 128, :],
                                y_sb[:])
                            done = q0 + (tt + 1) * 128
                            while rs_next[0] < len(RS_CHUNKS):
                                base, rows = RS_CHUNKS[rs_next[0]]
                                if base + rows > done:
                                    break
                                share = rows // 4
                                rs_out = drampool.tile(
                                    [share, D], bf16, tag=f"rs{rs_next[0]}")
                                nc.gpsimd.collective_compute(
                                    "ReduceScatter", mybir.AluOpType.add,
                                    replica_groups=GROUPS,
                                    ins=[y_bounce[base:base + rows, :]],
                                    outs=[rs_out[:]],
                                )
                                nc.sync.dma_start(
                                    out_ext.ap()[rs_next[1]:
                                                 rs_next[1] + share, :],
                                    rs_out[:])
                                rs_next[0] += 1
                                rs_next[1] += share

    nc.compile()
    return nc


def _install_profile_hook():
    """Provide antenv.axon_hooks (absent in this image) so bass_utils'
    axon trace path can reach the NTFF profiler in libaxon_pjrt.so."""
    try:
        import antenv
        if "antenv.axon_hooks" not in sys.modules:
            mod = types.ModuleType("antenv.axon_hooks")
            mod._hook = None
            mod.set_axon_ntff_profile_hook = lambda h: setattr(mod, "_hook", h)
            mod.get_axon_ntff_profile_hook = lambda: mod._hook
            sys.modules["antenv.axon_hooks"] = mod
            antenv.axon_hooks = mod
        from trn_agent_boot.trn_boot import _ntff_profile_via_ctypes
        hook = _ntff_profile_via_ctypes("/opt/axon/libaxon_pjrt.so")
        sys.modules["antenv.axon_hooks"].set_axon_ntff_profile_hook(hook)
        return True
    except Exception:
        return False


def kernel(x, W_qkv, W_proj):
    global LAST_EXEC_NS
    x = np.asarray(x, dtype=np.float32)
    W_qkv = np.asarray(W_qkv, dtype=np.float32)
    W_proj = np.asarray(W_proj, dtype=np.float32)

    if "nc" not in _CACHE:
        _CACHE["nc"] = _build()
    nc = _CACHE["nc"]

    npbf16 = mybir.dt.np(bf16)
    xT = [np.ascontiguousarray(x[b].T).astype(npbf16) for b in range(B)]
    in_maps = []
    for c in range(N_CORES):
        b, g = c // 4, c % 4
        wq = W_qkv[:, g * DSH:(g + 1) * DSH]
        wk = W_qkv[:, D + g * DSH:D + (g + 1) * DSH]
        wv = W_qkv[:, 2 * D + g * DSH:2 * D + (g + 1) * DSH]
        in_maps.append({
            "xT": xT[b],
            "w_qkv": np.concatenate([wq, wk, wv], axis=1).astype(npbf16),
            "w_proj": np.ascontiguousarray(
                W_proj[g * DSH:(g + 1) * DSH, :]).astype(npbf16),
        })

    profile = bool(os.environ.get("BASS_KERNEL_PROFILE"))
    trace_dir = os.environ.get("BASS_KERNEL_TRACE_DIR") or None
    if profile:
        profile = _install_profile_hook()
    res = run_bass_kernel_spmd(
        nc, in_maps, core_ids=list(range(N_CORES)),
        trace=profile, tmpdir=trace_dir)
    LAST_EXEC_NS = res.exec_time_ns

    y = np.empty((B, T, D), dtype=np.float32)
    for c in range(N_CORES):
        b, r = c // 4, c % 4
        oc = res.results[c]["out"].astype(np.float32)
        o = 0
        for base, rows in RS_CHUNKS:
            share = rows // 4
            y[b, base + r * share:base + (r + 1) * share, :] = oc[o:o + share]
            o += share
    return y



# revision 3
# speedup vs baseline: 1.0527x; 1.0527x over previous
"""Multi-head attention forward on 8 TRN2 NeuronCores.

Sharding: tensor-parallel over heads (4 groups of 4 heads) x data-parallel
over batch (2). Core c: batch c//4, heads [4*(c%4), 4*(c%4)+4).
Each 4-core batch group ReduceScatters the projection partials (bf16,
finer chunks at the tail so the last exposed collective is small) so every
core ends with disjoint [512, 1024] slices of the final output; the host
reassembles.

Compute layout is feature-major (transposed) throughout:
  qkvT = W_shard^T @ x^T          [768, T]   (PE, bf16 in / f32 psum)
  S^T  = kT^T qT per k-tile pair  [128, 1024] psum (two 512-col halves)
  P^T  = exp(S^T / 64)            (ScalarE, 1024 wide; no max-subtraction
                                   needed: scores have sigma ~0.125)
  O_aug^T = V_aug^T @ P^T accum   [65, 512]  (V_aug has a ones column so
                                   row 64 accumulates the softmax denom)
  epilogue: 1/denom on the [1,512] row (fast approx reciprocal), GpSimd
  partition-broadcast, DVE multiply -> O_all^T rows
  y = O_all^T^T @ W_proj          [128, 512] psum tiles

Schedule: the S->exp->O chain is software-pipelined (next pair's S before
previous pair's O), and ALL non-attention PE work (QKV blocks, V tiles,
proj tiles) is statically interleaved into the attention iterations as
filler so the PE never idles while ScalarE runs exp. Input DMAs are split
and priority-ordered so the first matmul starts ~6us in.
"""
import os
import sys
import types

import numpy as np

if "/opt/trn_rl_repo" not in sys.path:
    sys.path.insert(0, "/opt/trn_rl_repo")

import concourse.bass as bass
import concourse.bacc as bacc
import concourse.tile as tile
import concourse.mybir as mybir
from concourse import masks
from concourse.bass_utils import run_bass_kernel_spmd

B, T, D = 2, 2048, 1024
H, HD = 16, 64
N_CORES = 8
GROUPS = [[0, 1, 2, 3], [4, 5, 6, 7]]
HPC = 4                 # heads per core
DSH = HPC * HD          # 256 per-core head features
QKV_COLS = 3 * DSH      # 768
TQC = 512               # q-chunk
N_TQ = T // TQC         # 4
N_KT = T // 128         # 16 k-tiles
# reduce-scatter chunks (row_base, rows): coarse early, fine at the tail so
# the last exposed collective is small
RS_CHUNKS = ([(256 * i, 256) for i in range(6)]
             + [(1536 + 128 * i, 128) for i in range(3)]
             + [(1920, 64), (1984, 64)])

f32 = mybir.dt.float32
bf16 = mybir.dt.bfloat16

LAST_EXEC_NS = None
_CACHE = {}

# Static filler schedule: (tq, h, j) -> list of PE work units emitted after
# iteration j of attention head-chunk (tq, h).
#   ('v', i)        V k-tile pair kt=2i,2i+1 (4096 stream cols)
#   ('qkv', m, tc)  qkvT 128-row block m, t-chunk tc (4096 cols)
#   ('proj', tq, tt) projection rows [tq*512+tt*128, +128) (2048 cols)
# Deadlines: m2 tc c before S j=2c of hc(0,*); V pair i before O j=i of
# hc(0,0); m3 before hc(0,2); m0 tc c before hc(c,0); m1 tc c before
# hc(c,2); proj(tq) after hc(tq,3).
FILL = {
    (0, 0, 0): [('v', 1), ('qkv', 2, 1)],
    (0, 0, 1): [('v', 2)],
    (0, 0, 2): [('v', 3), ('qkv', 2, 2)],
    (0, 0, 3): [('v', 4)],
    (0, 0, 4): [('v', 5), ('qkv', 2, 3)],
    (0, 0, 5): [('v', 6)],
    (0, 0, 6): [('v', 7)],
    (0, 0, 7): [('qkv', 3, 0)],
    (0, 1, 1): [('qkv', 3, 1)],
    (0, 1, 3): [('qkv', 3, 2)],
    (0, 1, 5): [('qkv', 3, 3)],
    (0, 1, 7): [('qkv', 1, 0)],
    (0, 2, 1): [('qkv', 0, 1)],
    (0, 2, 5): [('qkv', 1, 1)],
    (1, 0, 1): [('proj', 0, 0)],
    (1, 0, 3): [('proj', 0, 1)],
    (1, 0, 5): [('proj', 0, 2)],
    (1, 0, 7): [('proj', 0, 3)],
    (1, 1, 1): [('qkv', 0, 2)],
    (1, 1, 5): [('qkv', 1, 2)],
    (2, 0, 1): [('proj', 1, 0)],
    (2, 0, 3): [('proj', 1, 1)],
    (2, 0, 5): [('proj', 1, 2)],
    (2, 0, 7): [('proj', 1, 3)],
    (2, 1, 1): [('qkv', 0, 3)],
    (2, 1, 5): [('qkv', 1, 3)],
    (3, 0, 1): [('proj', 2, 0)],
    (3, 0, 3): [('proj', 2, 1)],
    (3, 0, 5): [('proj', 2, 2)],
    (3, 0, 7): [('proj', 2, 3)],
}


def _build():
    nc = bacc.Bacc("TRN2", target_bir_lowering=False, debug=False,
                   num_devices=N_CORES)
    xT_ext = nc.dram_tensor("xT", [D, T], bf16, kind="ExternalInput")
    wqkv_ext = nc.dram_tensor("w_qkv", [D, QKV_COLS], bf16,
                              kind="ExternalInput")
    wproj_ext = nc.dram_tensor("w_proj", [DSH, D], bf16, kind="ExternalInput")
    out_ext = nc.dram_tensor("out", [T // 4, D], bf16, kind="ExternalOutput")
    Exp = mybir.ActivationFunctionType.Exp

    with tile.TileContext(nc) as tc:
        with (
            tc.tile_pool(name="persist", bufs=1) as persist,
            tc.tile_pool(name="dram", bufs=1, space="DRAM") as drampool,
            tc.tile_pool(name="ps_s", bufs=2, space="PSUM") as pss,
            tc.tile_pool(name="ps_o", bufs=2, space="PSUM") as pso,
            tc.tile_pool(name="ps_f", bufs=2, space="PSUM") as psf,
            tc.tile_pool(name="attn", bufs=3) as apool,
            tc.tile_pool(name="attn2", bufs=2) as apool2,
        ):
            qkvT = persist.tile([128, 4, T], bf16)   # q,k rows m*128+p of [512,T]
            wproj = persist.tile([128, 2, D], bf16)  # rows of [256, 1024]
            oallT = persist.tile([128, 2, T], bf16)  # rows of O_all^T [256, T]
            vaug = persist.tile([128, HPC, N_KT, 80], bf16)
            # per-t-chunk x tiles so DMA completion gates at chunk granularity
            xTc = [persist.tile([128, 8, TQC], bf16, name=f"xTc{i}")
                   for i in range(N_TQ)]
            wq_k = persist.tile([128, 8, 2 * 128], bf16)  # k cols (m=2,3)
            wq_q = persist.tile([128, 8, 2 * 128], bf16)  # q cols (m=0,1)
            wq_v = persist.tile([128, 8, DSH], bf16)      # v cols

            y_bounce = drampool.tile([T, D], bf16, tag="ybounce")

            # ---- input DMAs, priority-ordered for earliest first matmul ----
            wq_src = wqkv_ext.ap().rearrange("(k p) m -> p k m", p=128)
            xT_src = xT_ext.ap().rearrange("(k p) t -> p k t", p=128)
            nc.sync.dma_start(wq_k[:], wq_src[:, :, 256:512])
            nc.sync.dma_start(xTc[0][:], xT_src[:, :, 0:TQC])
            nc.sync.dma_start(wq_q[:], wq_src[:, :, 0:256])
            nc.sync.dma_start(wq_v[:], wq_src[:, :, 512:768])
            for tch in range(1, N_TQ):
                nc.sync.dma_start(xTc[tch][:],
                                  xT_src[:, :, tch * TQC:(tch + 1) * TQC])
            nc.sync.dma_start(
                wproj[:], wproj_ext.ap().rearrange("(c p) d -> p c d", p=128))
            nc.gpsimd.memset(vaug[:], 1.0)

            # ---- PE work units ----
            def qkv_m_tc(m, tch):
                """One 128-row block m of qkvT for one 512-col t-chunk."""
                w = wq_k if m >= 2 else wq_q
                mb = (m - 2 if m >= 2 else m) * 128
                t0 = tch * TQC
                ps = psf.tile([128, TQC], f32, tag="f")
                for k in range(8):
                    nc.tensor.matmul(
                        ps[:],
                        w[:, k, mb:mb + 128],
                        xTc[tch][:, k, :],
                        start=(k == 0), stop=(k == 7),
                    )
                nc.vector.tensor_copy(qkvT[:, m, t0:t0 + TQC], ps[:])

            def v_pair(i):
                """V (in [T, hd] orientation) for k-tiles 2i, 2i+1."""
                ps = psf.tile([128, TQC], f32, tag="f")
                for half in range(2):
                    kt = 2 * i + half
                    tch, tb = kt // 4, (kt % 4) * 128
                    base = half * DSH
                    for k in range(8):
                        nc.tensor.matmul(
                            ps[:, base:base + DSH],
                            xTc[tch][:, k, tb:tb + 128],
                            wq_v[:, k, :],
                            start=(k == 0), stop=(k == 7),
                        )
                for half in range(2):
                    kt = 2 * i + half
                    nc.vector.tensor_copy(
                        vaug[:, :, kt, 0:HD],
                        ps[:, half * DSH:(half + 1) * DSH].rearrange(
                            "p (h d) -> p h d", d=HD))

            rs_next = [0, 0]

            def rs_poll(done_rows):
                while rs_next[0] < len(RS_CHUNKS):
                    base, rows = RS_CHUNKS[rs_next[0]]
                    if base + rows > done_rows:
                        break
                    share = rows // 4
                    rs_out = drampool.tile(
                        [share, D], bf16, tag=f"rs{rs_next[0]}")
                    nc.gpsimd.collective_compute(
                        "ReduceScatter", mybir.AluOpType.add,
                        replica_groups=GROUPS,
                        ins=[y_bounce[base:base + rows, :]],
                        outs=[rs_out[:]],
                    )
                    nc.sync.dma_start(
                        out_ext.ap()[rs_next[1]:rs_next[1] + share, :],
                        rs_out[:])
                    rs_next[0] += 1
                    rs_next[1] += share

            def proj_tt(tq, tt):
                q0 = tq * TQC
                y_sb = apool.tile([128, D], bf16, tag="ysb")
                for nn2 in range(2):
                    y_ps = psf.tile([128, TQC], f32, tag="f")
                    for kc in range(2):
                        nc.tensor.matmul(
                            y_ps[:],
                            oallT[:, kc, q0 + tt * 128:q0 + (tt + 1) * 128],
                            wproj[:, kc, nn2 * 512:(nn2 + 1) * 512],
                            start=(kc == 0), stop=(kc == 1),
                        )
                    nc.vector.tensor_copy(
                        y_sb[:, nn2 * 512:(nn2 + 1) * 512], y_ps[:])
                nc.sync.dma_start(
                    y_bounce[q0 + tt * 128:q0 + (tt + 1) * 128, :], y_sb[:])
                rs_poll(q0 + (tt + 1) * 128)

            def run_unit(u):
                if u[0] == 'v':
                    v_pair(u[1])
                elif u[0] == 'qkv':
                    qkv_m_tc(u[1], u[2])
                else:
                    proj_tt(u[1], u[2])

            def epilogue(tq, h, o_sb, o_ps):
                """Normalize head h's O (feature-major) into oallT:
                reciprocal on the [1, TQC] denominator row (fast approx),
                GpSimd partition-broadcast, DVE multiply."""
                q0 = tq * TQC
                om, op = h // 2, (h % 2) * 64
                rrow = apool2.tile([1, TQC], f32, tag="rrow")
                nc.vector.tensor_copy(rrow[:], o_ps[HD:HD + 1, :])
                rinv = apool2.tile([1, TQC], f32, tag="rinv")
                nc.vector.reciprocal_approx_fast(out=rinv[:], in_=rrow[:])
                rb = apool2.tile([HD, TQC], f32, tag="rb")
                nc.gpsimd.partition_broadcast(rb[:], rinv[:])
                nc.vector.tensor_tensor(
                    out=oallT[op:op + HD, om, q0:q0 + TQC],
                    in0=o_sb[0:HD, :], in1=rb[:],
                    op=mybir.AluOpType.mult)

            def attn_unit(tq, h):
                q0 = tq * TQC
                qm, qp = h // 2, (h % 2) * 64
                km = 2 + h // 2
                o_ps = pso.tile([HD + 1, TQC], f32, tag="o")
                prev_p = None
                for j in range(N_KT // 2):   # k-tile pairs
                    s2 = pss.tile([128, 2 * TQC], f32, tag="s")
                    for half in range(2):
                        kt = 2 * j + half
                        nc.tensor.matmul(
                            s2[:, half * TQC:(half + 1) * TQC],
                            qkvT[qp:qp + HD, km, kt * 128:(kt + 1) * 128],
                            qkvT[qp:qp + HD, qm, q0:q0 + TQC],
                            start=True, stop=True,
                        )
                    p2 = apool.tile([128, 2 * TQC], bf16, tag="p")
                    nc.scalar.activation(p2[:], s2[:], Exp, scale=1.0 / HD)
                    if prev_p is not None:
                        pj, pp = prev_p
                        for half in range(2):
                            kt = 2 * pj + half
                            nc.tensor.matmul(
                                o_ps[:], vaug[:, h, kt, 0:HD + 1],
                                pp[:, half * TQC:(half + 1) * TQC],
                                start=(kt == 0), stop=False,
                            )
                    prev_p = (j, p2)
                    for u in FILL.get((tq, h, j), []):
                        run_unit(u)
                pj, pp = prev_p
                for half in range(2):
                    kt = 2 * pj + half
                    nc.tensor.matmul(
                        o_ps[:], vaug[:, h, kt, 0:HD + 1],
                        pp[:, half * TQC:(half + 1) * TQC],
                        start=False, stop=(kt == N_KT - 1),
                    )
                o_sb = apool.tile([HD + 1, TQC], bf16, tag="osb")
                nc.vector.tensor_copy(o_sb[:], o_ps[:])
                return o_sb, o_ps

            # ---- prefix: minimum PE work before attention can start ----
            qkv_m_tc(2, 0)   # k rows for heads 0,1; t-chunk 0
            qkv_m_tc(0, 0)   # q rows for heads 0,1; chunk 0
            v_pair(0)        # V k-tiles 0,1

            # ---- attention with interleaved filler ----
            for tq in range(N_TQ):
                for h in range(HPC):
                    o_sb, o_ps = attn_unit(tq, h)
                    epilogue(tq, h, o_sb, o_ps)

            # ---- tail: last chunk's proj + remaining reduce-scatters ----
            for tt in range(4):
                proj_tt(3, tt)

    nc.compile()
    return nc


def _install_profile_hook():
    """Provide antenv.axon_hooks (absent in this image) so bass_utils'
    axon trace path can reach the NTFF profiler in libaxon_pjrt.so."""
    try:
        import antenv
        if "antenv.axon_hooks" not in sys.modules:
            mod = types.ModuleType("antenv.axon_hooks")
            mod._hook = None
            mod.set_axon_ntff_profile_hook = lambda h: setattr(mod, "_hook", h)
            mod.get_axon_ntff_profile_hook = lambda: mod._hook
            sys.modules["antenv.axon_hooks"] = mod
            antenv.axon_hooks = mod
        from trn_agent_boot.trn_boot import _ntff_profile_via_ctypes
        hook = _ntff_profile_via_ctypes("/opt/axon/libaxon_pjrt.so")
        sys.modules["antenv.axon_hooks"].set_axon_ntff_profile_hook(hook)
        return True
    except Exception:
        return False


def kernel(x, W_qkv, W_proj):
    global LAST_EXEC_NS
    x = np.asarray(x, dtype=np.float32)
    W_qkv = np.asarray(W_qkv, dtype=np.float32)
    W_proj = np.asarray(W_proj, dtype=np.float32)

    if "nc" not in _CACHE:
        _CACHE["nc"] = _build()
    nc = _CACHE["nc"]

    npbf16 = mybir.dt.np(bf16)
    xT = [np.ascontiguousarray(x[b].T).astype(npbf16) for b in range(B)]
    in_maps = []
    for c in range(N_CORES):
        b, g = c // 4, c % 4
        wq = W_qkv[:, g * DSH:(g + 1) * DSH]
        wk = W_qkv[:, D + g * DSH:D + (g + 1) * DSH]
        wv = W_qkv[:, 2 * D + g * DSH:2 * D + (g + 1) * DSH]
        in_maps.append({
            "xT": xT[b],
            "w_qkv": np.concatenate([wq, wk, wv], axis=1).astype(npbf16),
            "w_proj": np.ascontiguousarray(
                W_proj[g * DSH:(g + 1) * DSH, :]).astype(npbf16),
        })

    profile = bool(os.environ.get("BASS_KERNEL_PROFILE"))
    trace_dir = os.environ.get("BASS_KERNEL_TRACE_DIR") or None
    if profile:
        profile = _install_profile_hook()
    res = run_bass_kernel_spmd(
        nc, in_maps, core_ids=list(range(N_CORES)),
        trace=profile, tmpdir=trace_dir)
    LAST_EXEC_NS = res.exec_time_ns

    y = np.empty((B, T, D), dtype=np.float32)
    for c in range(N_CORES):
        b, r = c // 4, c % 4
        oc = res.results[c]["out"].astype(np.float32)
        o = 0
        for base, rows in RS_CHUNKS:
            share = rows // 4
            y[b, base + r * share:base + (r + 1) * share, :] = oc[o:o + share]
            o += share
    return y


# revision 8
# speedup vs baseline: 1.0586x; 1.0056x over previous
"""Multi-head attention forward on 8 TRN2 NeuronCores.

Sharding: tensor-parallel over heads (4 groups of 4 heads) x data-parallel
over batch (2). Core c: batch c//4, heads [4*(c%4), 4*(c%4)+4).
Each 4-core batch group ReduceScatters the projection partials (bf16, 8
chunks of 256 rows, overlapped with compute) so every core ends with
disjoint [512, 1024] slices of the final output; the host reassembles.

Compute layout is feature-major (transposed) throughout:
  qkvT = W_shard^T @ x^T          [768, T]   (PE, bf16 in / f32 psum)
  S^T  = kT^T qT per k-tile pair  [128, 2*qw] psum
  P^T  = exp(S^T / 64)            (ScalarE; no max-subtraction needed:
                                   scores have sigma ~0.125)
  O_aug^T = V_aug^T @ P^T accum   [65, qw]   (V_aug has a ones column so
                                   row 64 accumulates the softmax denom)
  epilogue: approx-reciprocal of the [1, qw] denom row, GpSimd
  partition-broadcast, DVE multiply -> O_all^T rows
  y = O_all^T^T @ W_proj          [128, 512] psum tiles

Schedule: the S->exp->O chain is software-pipelined (next pair's S before
previous pair's O) and ALL non-attention PE work (QKV blocks, V tiles,
proj tiles) is statically interleaved into the attention iterations as
filler, ordered by input-DMA arrival so the in-order PE queue never
stalls. The last q-chunk is processed as two 256-wide halves so the
second-to-last ReduceScatter overlaps attention and only the final
256-row RS is exposed in the tail. y_bounce DMAs are issued from the DVE
queue (not Sync) so they are never stuck behind an out-DMA that waits on
the slow CC stream.
"""
import os
import sys
import types

import numpy as np

if "/opt/trn_rl_repo" not in sys.path:
    sys.path.insert(0, "/opt/trn_rl_repo")

import concourse.bass as bass
import concourse.bacc as bacc
import concourse.tile as tile
import concourse.mybir as mybir
from concourse import masks
from concourse.bass_utils import run_bass_kernel_spmd

B, T, D = 2, 2048, 1024
H, HD = 16, 64
N_CORES = 8
GROUPS = [[0, 1, 2, 3], [4, 5, 6, 7]]
HPC = 4                 # heads per core
DSH = HPC * HD          # 256 per-core head features
QKV_COLS = 3 * DSH      # 768
TQC = 512               # q-chunk
N_TQ = T // TQC         # 4
N_KT = T // 128         # 16 k-tiles
RS_CHUNKS = [(256 * i, 256) for i in range(8)]

f32 = mybir.dt.float32
bf16 = mybir.dt.bfloat16

LAST_EXEC_NS = None
_CACHE = {}

# Attention head-chunks in processing order: (key, q0, qw)
HCS = ([('t0', 0, TQC), ('t1', TQC, TQC), ('t2', 2 * TQC, TQC),
        ('t3a', 3 * TQC, 256), ('t3b', 3 * TQC + 256, 256)])

# Static filler schedule: (key, h, j) -> units emitted after iteration j of
# attention head-chunk (key, h).
#   ('v', i)       V k-tile pair kt=2i,2i+1 (4096 stream cols)
#   ('qkv', m, tc) qkvT 128-row block m, t-chunk tc (4096 cols)
#   ('proj', r)    projection rows [r*128, r*128+128) (2048 cols)
# Ordering constraints: m2 tc c before S j=2c of t0; V pair i before O j=i
# of (t0, h0); m3 before (t0, h2); m0/m1 tc c before the chunks that read q
# from t-chunk c; proj rows r after the epilogues covering them. Early
# units additionally ordered by x-chunk DMA arrival (x tc1/tc2/tc3 land at
# roughly 20/24/28 us).
FILL = {
    ('t0', 0, 0): [('v', 0), ('v', 1)],
    ('t0', 0, 1): [('qkv', 2, 1), ('qkv', 1, 0)],
    ('t0', 0, 2): [('qkv', 2, 2), ('v', 2)],
    ('t0', 0, 3): [('v', 3), ('v', 4)],
    ('t0', 0, 4): [('qkv', 2, 3), ('v', 5)],
    ('t0', 0, 5): [('v', 6)],
    ('t0', 0, 6): [('v', 7)],
    ('t0', 0, 7): [('qkv', 3, 0)],
    ('t0', 1, 0): [('qkv', 3, 1)],
    ('t0', 1, 2): [('qkv', 3, 2)],
    ('t0', 1, 4): [('qkv', 3, 3)],
    ('t0', 1, 6): [('qkv', 0, 1)],
    ('t0', 2, 1): [('qkv', 1, 1)],
    ('t1', 0, 1): [('proj', 0)],
    ('t1', 0, 3): [('proj', 1)],
    ('t1', 0, 5): [('proj', 2)],
    ('t1', 0, 7): [('proj', 3)],
    ('t1', 1, 1): [('qkv', 0, 2)],
    ('t1', 1, 5): [('qkv', 1, 2)],
    ('t2', 0, 1): [('proj', 4)],
    ('t2', 0, 3): [('proj', 5)],
    ('t2', 0, 5): [('proj', 6)],
    ('t2', 0, 7): [('proj', 7)],
    ('t2', 1, 1): [('qkv', 0, 3)],
    ('t2', 1, 5): [('qkv', 1, 3)],
    ('t3a', 0, 1): [('proj', 8)],
    ('t3a', 0, 3): [('proj', 9)],
    ('t3a', 0, 5): [('proj', 10)],
    ('t3a', 0, 7): [('proj', 11)],
    ('t3b', 0, 1): [('proj', 12)],
    ('t3b', 0, 3): [('proj', 13)],
}


def _build():
    nc = bacc.Bacc("TRN2", target_bir_lowering=False, debug=False,
                   num_devices=N_CORES)
    xT_ext = nc.dram_tensor("xT", [D, T], bf16, kind="ExternalInput")
    wqkv_ext = nc.dram_tensor("w_qkv", [D, QKV_COLS], bf16,
                              kind="ExternalInput")
    wproj_ext = nc.dram_tensor("w_proj", [DSH, D], bf16, kind="ExternalInput")
    out_ext = nc.dram_tensor("out", [T // 4, D], bf16, kind="ExternalOutput")
    Exp = mybir.ActivationFunctionType.Exp

    with tile.TileContext(nc) as tc:
        with (
            tc.tile_pool(name="persist", bufs=1) as persist,
            tc.tile_pool(name="dram", bufs=1, space="DRAM") as drampool,
            tc.tile_pool(name="ps_s", bufs=2, space="PSUM") as pss,
            tc.tile_pool(name="ps_o", bufs=2, space="PSUM") as pso,
            tc.tile_pool(name="ps_f", bufs=2, space="PSUM") as psf,
            tc.tile_pool(name="attn", bufs=3) as apool,
            tc.tile_pool(name="attn2", bufs=2) as apool2,
        ):
            qkvT = persist.tile([128, 4, T], bf16)   # q,k rows m*128+p of [512,T]
            wproj = persist.tile([128, 2, D], bf16)  # rows of [256, 1024]
            oallT = persist.tile([128, 2, T], bf16)  # rows of O_all^T [256, T]
            vaug = persist.tile([128, HPC, N_KT, 80], bf16)
            # per-t-chunk x tiles so DMA completion gates at chunk granularity
            xTc = [persist.tile([128, 8, TQC], bf16, name=f"xTc{i}")
                   for i in range(N_TQ)]
            wq_k = persist.tile([128, 8, 2 * 128], bf16)  # k cols (m=2,3)
            wq_q = persist.tile([128, 8, 2 * 128], bf16)  # q cols (m=0,1)
            wq_v = persist.tile([128, 8, DSH], bf16)      # v cols

            y_bounce = drampool.tile([T, D], bf16, tag="ybounce")

            # ---- input DMAs, priority-ordered for earliest first matmul ----
            wq_src = wqkv_ext.ap().rearrange("(k p) m -> p k m", p=128)
            xT_src = xT_ext.ap().rearrange("(k p) t -> p k t", p=128)
            nc.sync.dma_start(wq_k[:], wq_src[:, :, 256:512])
            nc.sync.dma_start(xTc[0][:], xT_src[:, :, 0:TQC])
            nc.sync.dma_start(wq_q[:], wq_src[:, :, 0:256])
            nc.sync.dma_start(wq_v[:], wq_src[:, :, 512:768])
            for tch in range(1, N_TQ):
                nc.sync.dma_start(xTc[tch][:],
                                  xT_src[:, :, tch * TQC:(tch + 1) * TQC])
            nc.sync.dma_start(
                wproj[:], wproj_ext.ap().rearrange("(c p) d -> p c d", p=128))
            nc.gpsimd.memset(vaug[:], 1.0)

            # ---- PE work units ----
            def qkv_m_tc(m, tch):
                """One 128-row block m of qkvT for one 512-col t-chunk."""
                w = wq_k if m >= 2 else wq_q
                mb = (m - 2 if m >= 2 else m) * 128
                t0 = tch * TQC
                ps = psf.tile([128, TQC], f32, tag="f")
                for k in range(8):
                    nc.tensor.matmul(
                        ps[:],
                        w[:, k, mb:mb + 128],
                        xTc[tch][:, k, :],
                        start=(k == 0), stop=(k == 7),
                    )
                nc.vector.tensor_copy(qkvT[:, m, t0:t0 + TQC], ps[:])

            def v_pair(i):
                """V (in [T, hd] orientation) for k-tiles 2i, 2i+1."""
                ps = psf.tile([128, TQC], f32, tag="f")
                for half in range(2):
                    kt = 2 * i + half
                    tch, tb = kt // 4, (kt % 4) * 128
                    base = half * DSH
                    for k in range(8):
                        nc.tensor.matmul(
                            ps[:, base:base + DSH],
                            xTc[tch][:, k, tb:tb + 128],
                            wq_v[:, k, :],
                            start=(k == 0), stop=(k == 7),
                        )
                for half in range(2):
                    kt = 2 * i + half
                    nc.vector.tensor_copy(
                        vaug[:, :, kt, 0:HD],
                        ps[:, half * DSH:(half + 1) * DSH].rearrange(
                            "p (h d) -> p h d", d=HD))

            rs_next = [0, 0]

            def rs_poll(done_rows):
                while rs_next[0] < len(RS_CHUNKS):
                    base, rows = RS_CHUNKS[rs_next[0]]
                    if base + rows > done_rows:
                        break
                    share = rows // 4
                    rs_out = drampool.tile(
                        [share, D], bf16, tag=f"rs{rs_next[0]}")
                    nc.gpsimd.collective_compute(
                        "ReduceScatter", mybir.AluOpType.add,
                        replica_groups=GROUPS,
                        ins=[y_bounce[base:base + rows, :]],
                        outs=[rs_out[:]],
                    )
                    nc.sync.dma_start(
                        out_ext.ap()[rs_next[1]:rs_next[1] + share, :],
                        rs_out[:])
                    rs_next[0] += 1
                    rs_next[1] += share

            pending_flush = []

            def proj_r(r):
                """Projection for output rows [r*128, (r+1)*128)."""
                r0 = r * 128
                y_sb = apool.tile([128, D], bf16, tag="ysb")
                for nn2 in range(2):
                    y_ps = psf.tile([128, TQC], f32, tag="f")
                    for kc in range(2):
                        nc.tensor.matmul(
                            y_ps[:],
                            oallT[:, kc, r0:r0 + 128],
                            wproj[:, kc, nn2 * 512:(nn2 + 1) * 512],
                            start=(kc == 0), stop=(kc == 1),
                        )
                    nc.vector.tensor_copy(
                        y_sb[:, nn2 * 512:(nn2 + 1) * 512], y_ps[:])

                # y_bounce DMA issued from ScalarE (not Sync, where it would
                # queue behind an out-DMA that waits on the slow CC stream),
                # and deferred one filler slot so the y_sb cast is done and
                # the exp stream never waits.
                def flush():
                    nc.scalar.dma_start(y_bounce[r0:r0 + 128, :], y_sb[:])
                    rs_poll(r0 + 128)
                pending_flush.append(flush)

            def run_unit(u):
                if u[0] == 'v':
                    v_pair(u[1])
                elif u[0] == 'qkv':
                    qkv_m_tc(u[1], u[2])
                else:
                    proj_r(u[1])

            def epilogue(key, q0, qw, h, o_sb, o_ps):
                """Normalize head h's O (feature-major) into oallT."""
                om, op = h // 2, (h % 2) * 64
                rrow = apool2.tile([1, TQC], f32, tag="rrow")
                nc.vector.tensor_copy(rrow[:, 0:qw], o_ps[HD:HD + 1, :])
                rinv = apool2.tile([1, TQC], f32, tag="rinv")
                nc.vector.reciprocal_approx_fast(
                    out=rinv[:, 0:qw], in_=rrow[:, 0:qw])
                rb = apool2.tile([HD, TQC], f32, tag="rb")
                nc.gpsimd.partition_broadcast(rb[:, 0:qw], rinv[:, 0:qw])
                nc.vector.tensor_tensor(
                    out=oallT[op:op + HD, om, q0:q0 + qw],
                    in0=o_sb[0:HD, :], in1=rb[:, 0:qw],
                    op=mybir.AluOpType.mult)

            def attn_unit(key, q0, qw, h):
                qm, qp = h // 2, (h % 2) * 64
                km = 2 + h // 2
                o_ps_full = pso.tile([HD + 1, TQC], f32, tag="o")
                o_ps = o_ps_full[:, 0:qw]
                prev_p = None
                for j in range(N_KT // 2):   # k-tile pairs
                    s2f = pss.tile([128, 2 * TQC], f32, tag="s")
                    s2 = s2f[:, 0:2 * qw]
                    for half in range(2):
                        kt = 2 * j + half
                        nc.tensor.matmul(
                            s2[:, half * qw:(half + 1) * qw],
                            qkvT[qp:qp + HD, km, kt * 128:(kt + 1) * 128],
                            qkvT[qp:qp + HD, qm, q0:q0 + qw],
                            start=True, stop=True,
                        )
                    p2f = apool.tile([128, 2 * TQC], bf16, tag="p")
                    p2 = p2f[:, 0:2 * qw]
                    nc.scalar.activation(p2, s2, Exp, scale=1.0 / HD)
                    if prev_p is not None:
                        pj, pp = prev_p
                        for half in range(2):
                            kt = 2 * pj + half
                            nc.tensor.matmul(
                                o_ps, vaug[:, h, kt, 0:HD + 1],
                                pp[:, half * qw:(half + 1) * qw],
                                start=(kt == 0), stop=False,
                            )
                    prev_p = (j, p2)
                    if pending_flush:
                        todo, pending_flush[:] = list(pending_flush), []
                        for f in todo:
                            f()
                    for u in FILL.get((key, h, j), []):
                        run_unit(u)
                pj, pp = prev_p
                for half in range(2):
                    kt = 2 * pj + half
                    nc.tensor.matmul(
                        o_ps, vaug[:, h, kt, 0:HD + 1],
                        pp[:, half * qw:(half + 1) * qw],
                        start=False, stop=(kt == N_KT - 1),
                    )
                o_sb_f = apool.tile([HD + 1, TQC], bf16, tag="osb")
                o_sb = o_sb_f[:, 0:qw]
                nc.vector.tensor_copy(o_sb, o_ps)
                return o_sb, o_ps

            # ---- prefix: minimum PE work before attention can start ----
            qkv_m_tc(2, 0)   # k rows for heads 0,1; t-chunk 0
            qkv_m_tc(0, 0)   # q rows for heads 0,1; chunk 0

            # ---- attention with interleaved filler ----
            for key, q0, qw in HCS:
                for h in range(HPC):
                    o_sb, o_ps = attn_unit(key, q0, qw, h)
                    epilogue(key, q0, qw, h, o_sb, o_ps)

            # ---- tail: last half-chunk's proj + final reduce-scatter ----
            for f in pending_flush:
                f()
            del pending_flush[:]
            proj_r(14)
            proj_r(15)
            for f in pending_flush:
                f()

    nc.compile()
    return nc


def _install_profile_hook():
    """Provide antenv.axon_hooks (absent in this image) so bass_utils'
    axon trace path can reach the NTFF profiler in libaxon_pjrt.so."""
    try:
        import antenv
        if "antenv.axon_hooks" not in sys.modules:
            mod = types.ModuleType("antenv.axon_hooks")
            mod._hook = None
            mod.set_axon_ntff_profile_hook = lambda h: setattr(mod, "_hook", h)
            mod.get_axon_ntff_profile_hook = lambda: mod._hook
            sys.modules["antenv.axon_hooks"] = mod
            antenv.axon_hooks = mod
        from trn_agent_boot.trn_boot import _ntff_profile_via_ctypes
        hook = _ntff_profile_via_ctypes("/opt/axon/libaxon_pjrt.so")
        sys.modules["antenv.axon_hooks"].set_axon_ntff_profile_hook(hook)
        return True
    except Exception:
        return False


def kernel(x, W_qkv, W_proj):
    global LAST_EXEC_NS
    x = np.asarray(x, dtype=np.float32)
    W_qkv = np.asarray(W_qkv, dtype=np.float32)
    W_proj = np.asarray(W_proj, dtype=np.float32)

    if "nc" not in _CACHE:
        _CACHE["nc"] = _build()
    nc = _CACHE["nc"]

    npbf16 = mybir.dt.np(bf16)
    xT = [np.ascontiguousarray(x[b].T).astype(npbf16) for b in range(B)]
    in_maps = []
    for c in range(N_CORES):
        b, g = c // 4, c % 4
        wq = W_qkv[:, g * DSH:(g + 1) * DSH]
        wk = W_qkv[:, D + g * DSH:D + (g + 1) * DSH]
        wv = W_qkv[:, 2 * D + g * DSH:2 * D + (g + 1) * DSH]
        in_maps.append({
            "xT": xT[b],
            "w_qkv": np.concatenate([wq, wk, wv], axis=1).astype(npbf16),
            "w_proj": np.ascontiguousarray(
                W_proj[g * DSH:(g + 1) * DSH, :]).astype(npbf16),
        })

    profile = bool(os.environ.get("BASS_KERNEL_PROFILE"))
    trace_dir = os.environ.get("BASS_KERNEL_TRACE_DIR") or None
    if profile:
        profile = _install_profile_hook()
    res = run_bass_kernel_spmd(
        nc, in_maps, core_ids=list(range(N_CORES)),
        trace=profile, tmpdir=trace_dir)
    LAST_EXEC_NS = res.exec_time_ns

    y = np.empty((B, T, D), dtype=np.float32)
    for c in range(N_CORES):
        b, r = c // 4, c % 4
        oc = res.results[c]["out"].astype(np.float32)
        o = 0
        for base, rows in RS_CHUNKS:
            share = rows // 4
            y[b, base + r * share:base + (r + 1) * share, :] = oc[o:o + share]
            o += share
    return y


# revision 23
# speedup vs baseline: 1.0844x; 1.0243x over previous
"""Multi-head attention forward on 8 TRN2 NeuronCores.

Sharding: tensor-parallel over heads (4 groups of 4 heads) x data-parallel
over batch (2). Core c: batch c//4, heads [4*(c%4), 4*(c%4)+4).
Each 4-core batch group ReduceScatters the projection partials (bf16, 8
chunks of 256 rows, overlapped with compute) so every core ends with
disjoint [512, 1024] slices of the final output; the host reassembles.

Compute layout is feature-major (transposed) throughout:
  qkvT = W_shard^T @ x^T          [768, T]   (PE, bf16 in / f32 psum)
  S^T  = kT^T qT per k-tile pair  [128, 2*qw] psum
  P^T  = exp(S^T / 64)            (ScalarE; no max-subtraction needed:
                                   scores have sigma ~0.125)
  O_aug^T = V_aug^T @ P^T accum   [65, qw]   (V_aug has a ones column so
                                   row 64 accumulates the softmax denom)
  epilogue: approx-reciprocal of the [1, qw] denom row, GpSimd
  partition-broadcast, DVE multiply -> O_all^T rows
  y = O_all^T^T @ W_proj          [128, 512] psum tiles

Schedule: the S->exp->O chain is software-pipelined (next pair's S before
previous pair's O) and ALL non-attention PE work (QKV blocks, V tiles,
proj tiles) is statically interleaved into the attention iterations as
filler, ordered by input-DMA arrival so the in-order PE queue never
stalls. The last q-chunk is processed as two 256-wide halves so the
second-to-last ReduceScatter overlaps attention and only the final
256-row RS is exposed in the tail. y_bounce DMAs are issued from the DVE
queue (not Sync) so they are never stuck behind an out-DMA that waits on
the slow CC stream.
"""
import os
import sys
import types

import numpy as np

if "/opt/trn_rl_repo" not in sys.path:
    sys.path.insert(0, "/opt/trn_rl_repo")

import concourse.bass as bass
import concourse.bacc as bacc
import concourse.tile as tile
import concourse.mybir as mybir
from concourse import masks
from concourse.bass_utils import run_bass_kernel_spmd

B, T, D = 2, 2048, 1024
H, HD = 16, 64
N_CORES = 8
GROUPS = [[0, 1, 2, 3], [4, 5, 6, 7]]
HPC = 4                 # heads per core
DSH = HPC * HD          # 256 per-core head features
QKV_COLS = 3 * DSH      # 768
TQC = 512               # q-chunk
N_TQ = T // TQC         # 4
N_KT = T // 128         # 16 k-tiles
RS_CHUNKS = [(256 * i, 256) for i in range(8)]

f32 = mybir.dt.float32
bf16 = mybir.dt.bfloat16

LAST_EXEC_NS = None
_CACHE = {}

# Attention head-chunks in processing order: (key, q0, qw)
HCS = [('t0', 0, TQC), ('t1', TQC, TQC), ('t2', 2 * TQC, TQC),
       ('t3', 3 * TQC, TQC)]

# Static filler schedule: (key, h, j) -> units emitted after iteration j of
# attention head-chunk (key, h).
#   ('v', i)       V k-tile pair kt=2i,2i+1 (4096 stream cols)
#   ('qkv', m, tc) qkvT 128-row block m, t-chunk tc (4096 cols)
#   ('proj', r)    projection rows [r*128, r*128+128) (2048 cols)
# Ordering constraints: m2 tc c before S j=2c of t0; V pair i before O j=i
# of (t0, h0); m3 before (t0, h2); m0/m1 tc c before the chunks that read q
# from t-chunk c; proj rows r after the epilogues covering them. Early
# units additionally ordered by x-chunk DMA arrival (x tc1/tc2/tc3 land at
# roughly 20/24/28 us).
FILL = {
    ('t0', 0, 0): [('v', 0), ('v', 1)],
    ('t0', 0, 1): [('qkv', 2, 1), ('qkv', 1, 0)],
    ('t0', 0, 2): [('qkv', 2, 2), ('v', 2)],
    ('t0', 0, 3): [('v', 3), ('v', 4)],
    ('t0', 0, 4): [('qkv', 2, 3), ('v', 5)],
    ('t0', 0, 5): [('v', 6)],
    ('t0', 0, 6): [('v', 7)],
    ('t0', 0, 7): [('qkv', 3, 0)],
    ('t0', 1, 0): [('qkv', 3, 1)],
    ('t0', 1, 2): [('qkv', 3, 2)],
    ('t0', 1, 4): [('qkv', 3, 3)],
    ('t0', 1, 6): [('qkv', 0, 1)],
    ('t0', 2, 1): [('qkv', 1, 1)],
    ('t1', 0, 1): [('proj', 0)],
    ('t1', 0, 3): [('proj', 1)],
    ('t1', 0, 5): [('proj', 2)],
    ('t1', 0, 7): [('proj', 3)],
    ('t1', 1, 1): [('qkv', 0, 2)],
    ('t1', 1, 5): [('qkv', 1, 2)],
    ('t2', 0, 1): [('proj', 4)],
    ('t2', 0, 3): [('proj', 5)],
    ('t2', 0, 5): [('proj', 6)],
    ('t2', 0, 7): [('proj', 7)],
    ('t2', 1, 1): [('qkv', 0, 3)],
    ('t2', 1, 5): [('qkv', 1, 3)],
    ('t3', 0, 1): [('proj', 8)],
    ('t3', 0, 3): [('proj', 9)],
    ('t3', 0, 5): [('proj', 10)],
    ('t3', 0, 7): [('proj', 11)],
}


def _build():
    nc = bacc.Bacc("TRN2", target_bir_lowering=False, debug=False,
                   num_devices=N_CORES)
    xT_ext = nc.dram_tensor("xT", [D, T], bf16, kind="ExternalInput")
    wqkv_ext = nc.dram_tensor("w_qkv", [D, QKV_COLS], bf16,
                              kind="ExternalInput")
    wproj_ext = nc.dram_tensor("w_proj", [DSH, D], bf16, kind="ExternalInput")
    out_ext = nc.dram_tensor("out", [T // 4, D], bf16, kind="ExternalOutput")
    Exp = mybir.ActivationFunctionType.Exp

    with tile.TileContext(nc) as tc:
        with (
            tc.tile_pool(name="persist", bufs=1) as persist,
            tc.tile_pool(name="dram", bufs=1, space="DRAM") as drampool,
            tc.tile_pool(name="ps_s", bufs=2, space="PSUM") as pss,
            tc.tile_pool(name="ps_o", bufs=2, space="PSUM") as pso,
            tc.tile_pool(name="ps_f", bufs=2, space="PSUM") as psf,
            tc.tile_pool(name="attn", bufs=3) as apool,
            tc.tile_pool(name="attn2", bufs=2) as apool2,
        ):
            qkvT = persist.tile([128, 4, T], bf16)   # q,k rows m*128+p of [512,T]
            wproj = persist.tile([128, 2, D], bf16)  # rows of [256, 1024]
            oallT = persist.tile([128, 2, T], bf16)  # rows of O_all^T [256, T]
            vaug = persist.tile([128, HPC, N_KT, 80], bf16)
            # per-t-chunk x tiles so DMA completion gates at chunk granularity
            xTc = [persist.tile([128, 8, TQC], bf16, name=f"xTc{i}")
                   for i in range(N_TQ)]
            wq_k = persist.tile([128, 8, 2 * 128], bf16)  # k cols (m=2,3)
            wq_q = persist.tile([128, 8, 2 * 128], bf16)  # q cols (m=0,1)
            wq_v = persist.tile([128, 8, DSH], bf16)      # v cols
            fp16 = mybir.dt.float16
            # 64-partition broadcast matmul operands: lhsT has a single 1.0
            # row so out[i, j] = rhs[0, j]; rhs rows 1..63 stay zeroed.
            ones64 = persist.tile([HD, HD], fp16)
            rinv64 = persist.tile([HD, TQC], fp16)

            y_bounce = drampool.tile([T, D], bf16, tag="ybounce")

            # ---- input DMAs, priority-ordered for earliest first matmul ----
            wq_src = wqkv_ext.ap().rearrange("(k p) m -> p k m", p=128)
            xT_src = xT_ext.ap().rearrange("(k p) t -> p k t", p=128)
            nc.sync.dma_start(wq_k[:], wq_src[:, :, 256:512])
            nc.sync.dma_start(xTc[0][:], xT_src[:, :, 0:TQC])
            nc.sync.dma_start(wq_q[:], wq_src[:, :, 0:256])
            nc.sync.dma_start(wq_v[:], wq_src[:, :, 512:768])
            for tch in range(1, N_TQ):
                nc.sync.dma_start(xTc[tch][:],
                                  xT_src[:, :, tch * TQC:(tch + 1) * TQC])
            nc.sync.dma_start(
                wproj[:], wproj_ext.ap().rearrange("(c p) d -> p c d", p=128))
            nc.gpsimd.memset(vaug[:], 1.0)
            nc.gpsimd.memset(ones64[:], 0.0)
            nc.gpsimd.memset(ones64[0:1, :], 1.0)
            nc.gpsimd.memset(rinv64[:], 0.0)

            # ---- PE work units ----
            def qkv_m_tc(m, tch):
                """One 128-row block m of qkvT for one 512-col t-chunk."""
                w = wq_k if m >= 2 else wq_q
                mb = (m - 2 if m >= 2 else m) * 128
                t0 = tch * TQC
                ps = psf.tile([128, TQC], f32, tag="f")
                for k in range(8):
                    nc.tensor.matmul(
                        ps[:],
                        w[:, k, mb:mb + 128],
                        xTc[tch][:, k, :],
                        start=(k == 0), stop=(k == 7),
                    )
                nc.vector.tensor_copy(qkvT[:, m, t0:t0 + TQC], ps[:])

            def v_pair(i):
                """V (in [T, hd] orientation) for k-tiles 2i, 2i+1."""
                ps = psf.tile([128, TQC], f32, tag="f")
                for half in range(2):
                    kt = 2 * i + half
                    tch, tb = kt // 4, (kt % 4) * 128
                    base = half * DSH
                    for k in range(8):
                        nc.tensor.matmul(
                            ps[:, base:base + DSH],
                            xTc[tch][:, k, tb:tb + 128],
                            wq_v[:, k, :],
                            start=(k == 0), stop=(k == 7),
                        )
                for half in range(2):
                    kt = 2 * i + half
                    nc.vector.tensor_copy(
                        vaug[:, :, kt, 0:HD],
                        ps[:, half * DSH:(half + 1) * DSH].rearrange(
                            "p (h d) -> p h d", d=HD))

            rs_next = [0, 0]

            def rs_poll(done_rows):
                while rs_next[0] < len(RS_CHUNKS):
                    base, rows = RS_CHUNKS[rs_next[0]]
                    if base + rows > done_rows:
                        break
                    share = rows // 4
                    rs_out = drampool.tile(
                        [share, D], bf16, tag=f"rs{rs_next[0]}")
                    nc.gpsimd.collective_compute(
                        "ReduceScatter", mybir.AluOpType.add,
                        replica_groups=GROUPS,
                        ins=[y_bounce[base:base + rows, :]],
                        outs=[rs_out[:]],
                    )
                    nc.sync.dma_start(
                        out_ext.ap()[rs_next[1]:rs_next[1] + share, :],
                        rs_out[:])
                    rs_next[0] += 1
                    rs_next[1] += share

            def proj_r(r):
                """Projection for output rows [r*128, (r+1)*128)."""
                r0 = r * 128
                y_sb = apool.tile([128, D], bf16, tag="ysb")
                for nn2 in range(2):
                    y_ps = psf.tile([128, TQC], f32, tag="f")
                    for kc in range(2):
                        nc.tensor.matmul(
                            y_ps[:],
                            oallT[:, kc, r0:r0 + 128],
                            wproj[:, kc, nn2 * 512:(nn2 + 1) * 512],
                            start=(kc == 0), stop=(kc == 1),
                        )
                    nc.vector.tensor_copy(
                        y_sb[:, nn2 * 512:(nn2 + 1) * 512], y_ps[:])
                nc.sync.dma_start(y_bounce[r0:r0 + 128, :], y_sb[:])
                rs_poll(r0 + 128)

            def run_unit(u):
                if u[0] == 'v':
                    v_pair(u[1])
                elif u[0] == 'qkv':
                    qkv_m_tc(u[1], u[2])
                else:
                    proj_r(u[1])

            pend_epi = [None]

            def flush_epi():
                if pend_epi[0] is not None:
                    f, pend_epi[0] = pend_epi[0], None
                    f()

            def epilogue_start(q0, qw, h, o_sb, o_ps):
                """Begin normalizing head h's O: reciprocal of the [1, qw]
                denominator row on DVE now; returns a finisher that
                broadcasts it across 64 partitions with a tiny PE matmul
                (overwriting o_ps rows 0:64) and multiplies into oallT.
                The finisher is deferred one attention iteration so the PE
                never waits on the DVE reciprocal chain."""
                om, op = h // 2, (h % 2) * 64
                rrow = apool2.tile([1, TQC], f32, tag="rrow")
                nc.vector.tensor_copy(rrow[:, 0:qw], o_ps[HD:HD + 1, :])
                rinv = apool2.tile([1, TQC], f32, tag="rinv")
                nc.vector.reciprocal_approx_fast(
                    out=rinv[:, 0:qw], in_=rrow[:, 0:qw])
                nc.vector.tensor_copy(rinv64[0:1, 0:qw], rinv[:, 0:qw])

                def fin():
                    nc.tensor.matmul(o_ps[0:HD, :], ones64[:],
                                     rinv64[:, 0:qw], start=True, stop=True)
                    nc.vector.tensor_tensor(
                        out=oallT[op:op + HD, om, q0:q0 + qw],
                        in0=o_sb[0:HD, :], in1=o_ps[0:HD, :],
                        op=mybir.AluOpType.mult)
                return fin

            def attn_unit(key, q0, qw, h):
                qm, qp = h // 2, (h % 2) * 64
                km = 2 + h // 2
                o_ps_full = pso.tile([HD + 1, TQC], f32, tag="o")
                o_ps = o_ps_full[:, 0:qw]
                prev_p = None
                for j in range(N_KT // 2):   # k-tile pairs
                    s2f = pss.tile([128, 2 * TQC], f32, tag="s")
                    s2 = s2f[:, 0:2 * qw]
                    for half in range(2):
                        kt = 2 * j + half
                        nc.tensor.matmul(
                            s2[:, half * qw:(half + 1) * qw],
                            qkvT[qp:qp + HD, km, kt * 128:(kt + 1) * 128],
                            qkvT[qp:qp + HD, qm, q0:q0 + qw],
                            start=True, stop=True,
                        )
                    p2f = apool.tile([128, 2 * TQC], bf16, tag="p")
                    p2 = p2f[:, 0:2 * qw]
                    nc.scalar.activation(p2, s2, Exp, scale=1.0 / HD)
                    if prev_p is not None:
                        pj, pp = prev_p
                        for half in range(2):
                            kt = 2 * pj + half
                            nc.tensor.matmul(
                                o_ps, vaug[:, h, kt, 0:HD + 1],
                                pp[:, half * qw:(half + 1) * qw],
                                start=(kt == 0), stop=False,
                            )
                    prev_p = (j, p2)
                    for u in FILL.get((key, h, j), []):
                        run_unit(u)
                    if j == 0:
                        flush_epi()
                pj, pp = prev_p
                for half in range(2):
                    kt = 2 * pj + half
                    nc.tensor.matmul(
                        o_ps, vaug[:, h, kt, 0:HD + 1],
                        pp[:, half * qw:(half + 1) * qw],
                        start=False, stop=(kt == N_KT - 1),
                    )
                o_sb_f = apool.tile([HD + 1, TQC], bf16, tag="osb")
                o_sb = o_sb_f[:, 0:qw]
                nc.vector.tensor_copy(o_sb, o_ps)
                return o_sb, o_ps

            # ---- prefix: minimum PE work before attention can start ----
            qkv_m_tc(2, 0)   # k rows for heads 0,1; t-chunk 0
            qkv_m_tc(0, 0)   # q rows for heads 0,1; chunk 0

            # ---- attention with interleaved filler ----
            for key, q0, qw in HCS:
                for h in range(HPC):
                    o_sb, o_ps = attn_unit(key, q0, qw, h)
                    pend_epi[0] = epilogue_start(q0, qw, h, o_sb, o_ps)

            # ---- tail: last chunk's proj + final reduce-scatters ----
            flush_epi()
            for r in (12, 13, 14, 15):
                proj_r(r)

    nc.compile()
    return nc


def _install_profile_hook():
    """Provide antenv.axon_hooks (absent in this image) so bass_utils'
    axon trace path can reach the NTFF profiler in libaxon_pjrt.so."""
    try:
        import antenv
        if "antenv.axon_hooks" not in sys.modules:
            mod = types.ModuleType("antenv.axon_hooks")
            mod._hook = None
            mod.set_axon_ntff_profile_hook = lambda h: setattr(mod, "_hook", h)
            mod.get_axon_ntff_profile_hook = lambda: mod._hook
            sys.modules["antenv.axon_hooks"] = mod
            antenv.axon_hooks = mod
        from trn_agent_boot.trn_boot import _ntff_profile_via_ctypes
        hook = _ntff_profile_via_ctypes("/opt/axon/libaxon_pjrt.so")
        sys.modules["antenv.axon_hooks"].set_axon_ntff_profile_hook(hook)
        return True
    except Exception:
        return False


def kernel(x, W_qkv, W_proj):
    global LAST_EXEC_NS
    x = np.asarray(x, dtype=np.float32)
    W_qkv = np.asarray(W_qkv, dtype=np.float32)
    W_proj = np.asarray(W_proj, dtype=np.float32)

    if "nc" not in _CACHE:
        _CACHE["nc"] = _build()
    nc = _CACHE["nc"]

    npbf16 = mybir.dt.np(bf16)
    xT = [np.ascontiguousarray(x[b].T).astype(npbf16) for b in range(B)]
    in_maps = []
    for c in range(N_CORES):
        b, g = c // 4, c % 4
        wq = W_qkv[:, g * DSH:(g + 1) * DSH]
        wk = W_qkv[:, D + g * DSH:D + (g + 1) * DSH]
        wv = W_qkv[:, 2 * D + g * DSH:2 * D + (g + 1) * DSH]
        in_maps.append({
            "xT": xT[b],
            "w_qkv": np.concatenate([wq, wk, wv], axis=1).astype(npbf16),
            "w_proj": np.ascontiguousarray(
                W_proj[g * DSH:(g + 1) * DSH, :]).astype(npbf16),
        })

    profile = bool(os.environ.get("BASS_KERNEL_PROFILE"))
    trace_dir = os.environ.get("BASS_KERNEL_TRACE_DIR") or None
    if profile:
        profile = _install_profile_hook()
    res = run_bass_kernel_spmd(
        nc, in_maps, core_ids=list(range(N_CORES)),
        trace=profile, tmpdir=trace_dir)
    LAST_EXEC_NS = res.exec_time_ns

    y = np.empty((B, T, D), dtype=np.float32)
    for c in range(N_CORES):
        b, r = c // 4, c % 4
        oc = res.results[c]["out"].astype(np.float32)
        o = 0
        for base, rows in RS_CHUNKS:
            share = rows // 4
            y[b, base + r * share:base + (r + 1) * share, :] = oc[o:o + share]
            o += share
    return y
